# revision 6
# baseline (speedup 1.0000x reference)
"""Trainium2 Bass kernel for the LDE1D vq_codebook problem, v7.

v6 + instruction grouping: per-op fixed costs (~100-180ns on ACT/DVE)
dominated v6's schedule, so the softmax chain now processes G=4 token
tiles per instruction. The u[k] multiply is folded into the q matmul
group as a +ln(u[k]) bias row (PE rank-1 matmul, last in the PSUM
accumulation group — bank pending-zero semantics make this exact).
accT0/accT1/nacc share one PSUM bank (single accumulation group).
PSUM->SBUF copy of x^T is split DVE/ACT by column range.

Stages per 4-tile group (offsets in groups): A+0 PE 8 transposes |
B+1 DVE+ACT copy | C+2 PE 8 q-matmuls + lnu row | D+3 ACT exp |
E+4 DVE reduce/recip/scl | F+5 Pool w-scale | G+6 PE 12 acc matmuls.

Math identical to v4 (see kernel_v4 docstring); host epilogue
e[k,d] = accT[d,k]/nacc[k] - mu[k,d].
"""

import sys
from contextlib import ExitStack

import numpy as np

sys.path.insert(0, "/opt/trn_rl_repo")

import ml_dtypes

import concourse.bass as bass
import concourse.tile as tile
from concourse import bacc, mybir
from concourse.bass_utils import run_bass_kernel_spmd

BF16 = mybir.dt.bfloat16
F32 = mybir.dt.float32

B, T, D, K = 64, 4096, 256, 64
NCORES = 8
BPC = B // NCORES
TT = 128
G = 4                       # tiles per group
CSPLIT = 176                # copy column split: DVE [0:CSPLIT], ACT rest

OFF_A, OFF_B, OFF_C, OFF_D, OFF_E, OFF_F, OFF_G = 0, 1, 2, 3, 4, 5, 6
DRAIN = OFF_G + 1


def build_program(bpc=BPC, t=T, reps=1, trn_type="TRN2"):
    ntiles = t // TT
    assert ntiles % G == 0
    nc = bacc.Bacc(trn_type, target_bir_lowering=False, debug=False,
                   num_devices=NCORES)
    x_d = nc.dram_tensor("x", [bpc, TT, ntiles, D], BF16,
                         kind="ExternalInput").ap()
    wsT_d = nc.dram_tensor("wsT", [TT, bpc * ntiles], F32,
                           kind="ExternalInput").ap()
    muT2_d = nc.dram_tensor("muT2", [128, 2 * K], BF16,
                            kind="ExternalInput").ap()
    lnu_d = nc.dram_tensor("lnu", [1, G * K], BF16, kind="ExternalInput").ap()
    ident_d = nc.dram_tensor("ident", [128, 128], BF16,
                             kind="ExternalInput").ap()
    accT_d = nc.dram_tensor("accT", [bpc, 128, 2, K], F32,
                            kind="ExternalOutput").ap()
    nacc_d = nc.dram_tensor("nacc", [1, bpc * K], F32,
                            kind="ExternalOutput").ap()

    with tile.TileContext(nc) as tc, ExitStack() as ctx:
        _body(ctx, tc, accT_d, nacc_d, x_d, wsT_d, muT2_d, lnu_d, ident_d,
              bpc, ntiles, reps)
    nc.compile()
    return nc


def _body(ctx, tc, accT_d, nacc_d, x_d, wsT_d, muT2_d, lnu_d, ident_d,
          bpc, ntiles, reps):
    nc = tc.nc
    ngroups = ntiles // G
    xb_pool = ctx.enter_context(tc.tile_pool(name="xb", bufs=3))
    const = ctx.enter_context(tc.tile_pool(name="const", bufs=1))
    muT2 = const.tile([128, 2 * K], BF16)
    lnu = const.tile([1, G * K], BF16)
    ident = const.tile([128, 128], BF16)
    ones = const.tile([TT, K], BF16)
    nc.gpsimd.memset(ones[:], 1.0)
    ones1 = const.tile([1, TT], BF16)
    nc.gpsimd.memset(ones1[:], 1.0)
    wsall = const.tile([TT, bpc * ntiles], F32)
    naccs = const.tile([1, bpc * K], F32)
    # batch 0's x sub-DMAs interleaved with const loads so the pipeline
    # fills as early as possible (SP queue is in-order; consts-first
    # would delay the first transposes by ~4us)
    nsplit0 = min(8, ntiles)
    q0 = ntiles // nsplit0
    xbt0 = xb_pool.tile([TT, ntiles, D], BF16, name="xbt0")

    def _x0(hh):
        nc.sync.dma_start(xbt0[:, hh * q0:(hh + 1) * q0, :],
                          x_d[0][:, hh * q0:(hh + 1) * q0, :])

    _x0(0)
    nc.sync.dma_start(ident[:], ident_d[:])
    if nsplit0 > 1:
        _x0(1)
    nc.sync.dma_start(muT2[:], muT2_d[:])
    nc.sync.dma_start(lnu[:], lnu_d[:])
    if nsplit0 > 2:
        _x0(2)
    nc.sync.dma_start(wsall[:], wsT_d[:])
    for hh in range(3, nsplit0):
        _x0(hh)

    xt_pool = ctx.enter_context(tc.tile_pool(name="xt", bufs=3))
    p_pool = ctx.enter_context(tc.tile_pool(name="p", bufs=3))
    w_pool = ctx.enter_context(tc.tile_pool(name="w", bufs=3))
    dt_pool = ctx.enter_context(tc.tile_pool(name="dt", bufs=3))
    rd_pool = ctx.enter_context(tc.tile_pool(name="rd", bufs=3))
    scl_pool = ctx.enter_context(tc.tile_pool(name="scl", bufs=3))
    res_pool = ctx.enter_context(tc.tile_pool(name="res", bufs=2))
    pt_psum = ctx.enter_context(tc.tile_pool(name="pt", bufs=3, space="PSUM"))
    pq_psum = ctx.enter_context(tc.tile_pool(name="pq", bufs=3, space="PSUM"))
    pa_psum = ctx.enter_context(tc.tile_pool(name="pa", bufs=2, space="PSUM"))

    nbat = reps * bpc
    ntotg = nbat * ngroups
    xb = {0: xbt0}
    pt_t, xt_t, pq_t, p_t, w_t, rd_t, scl_t = {}, {}, {}, {}, {}, {}, {}
    accb = {}
    next_nb = 1

    for it in range(ntotg + DRAIN):
        while next_nb < nbat and next_nb * ngroups <= it + 10:
            xbt = xb_pool.tile([TT, ntiles, D], BF16)
            nsplit = min(8, ntiles)
            q4 = ntiles // nsplit
            for hh in range(nsplit):
                nc.sync.dma_start(
                    xbt[:, hh * q4:(hh + 1) * q4, :],
                    x_d[next_nb % bpc][:, hh * q4:(hh + 1) * q4, :])
            xb[next_nb] = xbt
            next_nb += 1

        gg = it - OFF_A
        if 0 <= gg < ntotg:  # A: PE transposes (8 per group)
            nb, g = gg // ngroups, gg % ngroups
            pt = pt_psum.tile([128, G, 256], BF16)
            for j in range(G):
                xin = xb[nb][:, g * G + j, :]
                nc.tensor.transpose(pt[:, j, 0:128], xin[:, 0:128], ident[:])
                nc.tensor.transpose(pt[:, j, 128:256], xin[:, 128:256],
                                    ident[:])
            pt_t[gg] = pt

        gg = it - OFF_B
        if 0 <= gg < ntotg:  # B: copy PSUM->SBUF split DVE/ACT
            pt = pt_t.pop(gg)
            xt = xt_pool.tile([128, G, 256], BF16)
            nc.vector.tensor_copy(xt[:, :, 0:CSPLIT], pt[:, :, 0:CSPLIT])
            nc.scalar.copy(xt[:, :, CSPLIT:256], pt[:, :, CSPLIT:256])
            xt_t[gg] = xt

        gg = it - OFF_C
        if 0 <= gg < ntotg:  # C: PE q matmuls, one PSUM group + lnu row
            xt = xt_t.pop(gg)
            pq = pq_psum.tile([TT, G, K], F32)
            for j in range(G):
                nc.tensor.matmul(pq[:, j, :], xt[:, j, 0:128], muT2[:, 0:K],
                                 start=(j == 0), stop=False)
                nc.tensor.matmul(pq[:, j, :], xt[:, j, 128:256],
                                 muT2[:, K:2 * K], start=False, stop=False)
            nc.tensor.matmul(pq[:], ones1[:], lnu[:],
                             start=False, stop=True)
            pq_t[gg] = pq

        gg = it - OFF_D
        if 0 <= gg < ntotg:  # D: ACT exp (includes u via lnu bias row)
            p = p_pool.tile([TT, G, K], BF16)
            nc.scalar.activation(p[:], pq_t.pop(gg)[:],
                                 mybir.ActivationFunctionType.Exp)
            p_t[gg] = p

        gg = it - OFF_E
        if 0 <= gg < ntotg:  # E: DVE dt, rd, scl
            nb, g = gg // ngroups, gg % ngroups
            p = p_t[gg]
            dt = dt_pool.tile([TT, G], F32)
            nc.vector.tensor_reduce(dt[:], p[:], mybir.AxisListType.X,
                                    mybir.AluOpType.add)
            rd = rd_pool.tile([TT, G], F32)
            nc.vector.reciprocal(rd[:], dt[:])
            scl = scl_pool.tile([TT, G, 1], F32)
            col = (nb % bpc) * ntiles + g * G
            nc.vector.tensor_tensor(
                scl[:, :, 0], wsall[:, col:col + G], rd[:],
                mybir.AluOpType.mult)
            scl_t[gg] = scl

        gg = it - OFF_F
        if 0 <= gg < ntotg:  # F: Pool w = p * scl (broadcast over k)
            p = p_t.pop(gg)
            scl = scl_t.pop(gg)
            w = w_pool.tile([TT, G, K], BF16)
            sb, wb = bass.broadcast_tensor_aps(scl[:], w[:])
            nc.gpsimd.tensor_tensor(w[:], p[:], sb, mybir.AluOpType.mult)
            w_t[gg] = w

        gg = it - OFF_G
        if 0 <= gg < ntotg:  # G: PE acc matmuls (+ batch epilogue)
            nb, g = gg // ngroups, gg % ngroups
            if g == 0:
                accb[nb] = pa_psum.tile([128, 3 * K], F32, name="accb")
            ab = accb[nb]
            w = w_t.pop(gg)
            for j in range(G):
                ti = g * G + j
                first = ti == 0
                last = ti == ntiles - 1
                xin = xb[nb][:, ti, :]
                wj = w[:, j, :]
                nc.tensor.matmul(ab[:, 0:K], xin[:, 0:128], wj,
                                 start=first, stop=last,
                                 skip_group_check=True)
                nc.tensor.matmul(ab[:, K:2 * K], xin[:, 128:256], wj,
                                 start=False, stop=last,
                                 skip_group_check=True)
                nc.tensor.matmul(ab[0:K, 2 * K:3 * K], ones[:, 0:K], wj,
                                 start=False, stop=last,
                                 skip_group_check=True)
            if g == ngroups - 1:
                b = nb % bpc
                accs = res_pool.tile([128, 2, K], F32, tag="accs")
                nc.vector.tensor_copy(accs[:, 0, :], ab[:, 0:K])
                nc.scalar.copy(accs[:, 1, :], ab[:, K:2 * K])
                nc.scalar.copy(naccs[:, b * K:(b + 1) * K],
                               ab[0:1, 2 * K:3 * K])
                nc.sync.dma_start(accT_d[b], accs[:])
                del accb[nb], xb[nb]
    nc.sync.dma_start(nacc_d[:], naccs[:])


def make_inputs(x, weights, mu, s, bpc=BPC, t=T):
    """Host-side prep: shard + precompute small replicated tensors."""
    ntiles = t // TT
    s = np.asarray(s, dtype=np.float32)
    s0 = float(s[0])
    if not np.allclose(s, s0):
        raise NotImplementedError("kernel assumes uniform s (as in setup)")
    mu = np.ascontiguousarray(mu, dtype=np.float32)
    mu2t = (2.0 * s0 * mu).T.astype(ml_dtypes.bfloat16)      # [D, K]
    muT2 = np.concatenate([mu2t[:128], mu2t[128:]], axis=1)  # [128, 2K]
    c = s0 * np.sum(mu.astype(np.float64) ** 2, axis=1)
    lnu = np.tile(-c.astype(np.float32), G).reshape(1, G * K)
    lnu = lnu.astype(ml_dtypes.bfloat16)
    ident = np.eye(128, dtype=ml_dtypes.bfloat16)
    ncores = x.shape[0] // bpc
    # one fused pass over x: cast fp32->bf16 + tile-permute into the
    # DMA layout [B, TT, ntiles, D] (numpy casts during assignment)
    xbf = np.empty((x.shape[0], TT, ntiles, D), dtype=ml_dtypes.bfloat16)
    xbf[...] = x[:, :t].reshape(x.shape[0], ntiles, TT, D).transpose(
        0, 2, 1, 3)
    ws = np.asarray(weights[:, :t], dtype=np.float32)
    wsT = ws.reshape(x.shape[0], ntiles, TT).transpose(2, 0, 1)
    in_maps = []
    for ci in range(ncores):
        sl = slice(ci * bpc, (ci + 1) * bpc)
        in_maps.append({
            "x": xbf[sl],
            "wsT": np.ascontiguousarray(
                wsT[:, sl].reshape(TT, bpc * ntiles)),
            "muT2": muT2, "lnu": lnu, "ident": ident,
        })
    return in_maps


OUTPUT_NAMES = ["accT", "nacc"]


def postprocess(outs, mu, bpc=BPC):
    accT = outs["accT"]                         # [bpc, 128, 2, K]
    nacc = outs["nacc"].reshape(bpc, K)
    A = accT.transpose(0, 3, 2, 1).reshape(bpc, K, D)   # [b, k, d]
    e = A / nacc.reshape(bpc, K, 1) - mu[None]
    return e.reshape(bpc, K * D).astype(np.float32)


_CACHE = {}


def _get_program():
    if "nc" not in _CACHE:
        _CACHE["nc"] = build_program()
    return _CACHE["nc"]


def kernel(x, weights, mu, s):
    x = np.asarray(x)
    weights = np.asarray(weights)
    mu = np.asarray(mu, dtype=np.float32)
    s = np.asarray(s, dtype=np.float32)
    nc = _get_program()
    in_maps = make_inputs(x, weights, mu, s)
    res = run_bass_kernel_spmd(nc, in_maps, core_ids=list(range(NCORES)))
    outs = [postprocess(res.results[ci], mu) for ci in range(NCORES)]
    return np.concatenate(outs, axis=0)


if __name__ == "__main__":
    rng = np.random.default_rng(0)
    x = rng.standard_normal((B, T, D), dtype=np.float32)
    w = rng.random((B, T), dtype=np.float32)
    mu = (0.1 * rng.standard_normal((K, D))).astype(np.float32)
    s = np.ones((K,), dtype=np.float32)
    out = kernel(x, weights=w, mu=mu, s=s)
    print("out", out.shape, out.dtype)
